# revision 10
# baseline (speedup 1.0000x reference)
"""Trainium2 Bass kernel for 12-head causal MHA (B=4, S=2048, D=768).

Sharding: 8 cores = 4 batches x 2 head-groups (6 heads each).
Device per core: project qT/kT (feature-major) + v (row-major, head-pair
augmented with ones cols); dual score computation:
  - natural [q, kv] pass -> qk logits output (causal-valid region only)
  - transposed [kv, q] pass -> exp -> masked -> [v|1]^T E matmul accumulating
    numerator + Z together in PSUM -> reciprocal * numerator = attn output
    directly in the transposed layout needed for the Wo projection.
All matmuls in float32r (full-rate fp32).
Host: shard prep (transpose x, slice/scale weights), sum the two Wo partial
products per batch + bo, and fill the constant causal -inf region of qk.
"""

import os
import numpy as np

import concourse.bacc as bacc
import concourse.mybir as mybir
import concourse.tile as tile
import concourse.ap as ap_mod
from concourse.bass_utils import run_bass_kernel_spmd

F32 = mybir.dt.float32
F32R = mybir.dt.float32r
AF = mybir.ActivationFunctionType
OP = mybir.AluOpType

B, D, H, HD = 4, 768, 12, 64
S = int(os.environ.get("K_S", "2048"))
HC = 6            # heads per core
C = HC * HD       # 384 cols per core
KT = D // 128     # 6 feature tiles
CT = C // 128     # 3 col tiles
NT = S // 128     # row tiles
G = S // 512      # q supertiles
NQ = S // 512     # xT quarters (512 rows each)

_CACHED_NC = None


def _ap3(base, offset, pattern):
    return ap_mod.AP(tensor=base.tensor, offset=base.offset + offset, ap=pattern)


def build_nc():
    PH = int(os.environ.get("K_PHASES", "3"))
    MASK_DVE = os.environ.get("K_MASK_DVE", "0") == "1"
    SKIP2 = os.environ.get("K_SKIP2", "0") == "1"
    nc = bacc.Bacc(trn_type="TRN2", target_bir_lowering=False, debug=False)

    xT = nc.dram_tensor("xT", [D, S], F32, kind="ExternalInput").ap()
    wq = nc.dram_tensor("wq", [D, C], F32, kind="ExternalInput").ap()
    wk = nc.dram_tensor("wk", [D, C], F32, kind="ExternalInput").ap()
    wv = nc.dram_tensor("wv", [D, C], F32, kind="ExternalInput").ap()
    bq = nc.dram_tensor("bq", [C, 1], F32, kind="ExternalInput").ap()
    bv = nc.dram_tensor("bv", [1, C], F32, kind="ExternalInput").ap()
    wo = nc.dram_tensor("wo", [C, D], F32, kind="ExternalInput").ap()
    masks = nc.dram_tensor("masks", [128, 4 * 512], F32, kind="ExternalInput").ap()
    one_row = nc.dram_tensor("one_row", [1, 128], F32, kind="ExternalInput").ap()

    qk = nc.dram_tensor("qk", [HC, S, S], F32, kind="ExternalOutput").ap()
    out = nc.dram_tensor("out", [S, D], F32, kind="ExternalOutput").ap()

    with tile.TileContext(nc) as tc:
        with tc.tile_pool(name="persist", bufs=1) as P:
            qT = P.tile([128, CT, S], F32R, tag="qT")
            kT = P.tile([128, CT, S], F32R, tag="kT")
            # v per row-tile, per head-pair: [vA(64) | ones(64) | vB(64)]
            vS = P.tile([128, NT, 3, 192], F32R, tag="vS")
            attT = P.tile([128, CT, S], F32R, tag="attT")
            woS = P.tile([128, CT, D], F32R, tag="woS")
            maskS = P.tile([128, 4, 512], F32R, tag="maskS")
            bqS = P.tile([128, CT], F32, tag="bqS")
            bvS = P.tile([1, C], F32R, tag="bvS")
            oneRow = P.tile([1, 128], F32R, tag="oneRow")
            onesT = P.tile([128, 64], F32, tag="onesT")

            nc.sync.dma_start(woS[:], wo.rearrange("(ct p) d -> p ct d", p=128).bitcast(F32R))
            nc.sync.dma_start(maskS[:], masks.rearrange("p (m f) -> p m f", f=512).bitcast(F32R))
            nc.sync.dma_start(bqS[:], bq.rearrange("(ct p) one -> p (ct one)", p=128))
            nc.sync.dma_start(bvS[:], bv.bitcast(F32R))
            nc.sync.dma_start(oneRow[:], one_row.bitcast(F32R))
            nc.vector.memset(onesT[:], 1.0)

            # ---------------- phase 1: projections ----------------
            with tc.tile_pool(name="xw", bufs=1) as PX, \
                 tc.tile_pool(name="wqkv", bufs=1) as PW, \
                 tc.tile_pool(name="ppq", bufs=2, space="PSUM") as PPQ, \
                 tc.tile_pool(name="ppv", bufs=2, space="PSUM") as PPV:
                wqS = PW.tile([128, KT, C], F32R, tag="wqS")
                wkS = PW.tile([128, KT, C], F32R, tag="wkS")
                wvS = PW.tile([128, KT, C], F32R, tag="wvS")
                nc.sync.dma_start(wqS[:], wq.rearrange("(k p) c -> p k c", p=128).bitcast(F32R))
                nc.sync.dma_start(wkS[:], wk.rearrange("(k p) c -> p k c", p=128).bitcast(F32R))
                nc.sync.dma_start(wvS[:], wv.rearrange("(k p) c -> p k c", p=128).bitcast(F32R))

                # ones columns of vS (positions 64..128 within each 192 block)
                for rt in range(NT):
                    dst = _ap3(vS[:, rt, 0, 0], 64,
                               [list(vS[:].ap[0]), [192, 3], [1, 64]])
                    src = _ap3(onesT[:, 0], 0,
                               [list(onesT[:].ap[0]), [0, 3], [1, 64]])
                    nc.vector.tensor_copy(dst.bitcast(F32R), src.bitcast(F32R))

                for quarter in range(NQ):
                    xTq = PX.tile([128, KT, 512], F32R, tag="xTq")
                    nc.sync.dma_start(
                        xTq[:],
                        xT[:, quarter * 512:(quarter + 1) * 512]
                        .rearrange("(k p) s -> p k s", p=128).bitcast(F32R))
                    # qT / kT projections for this 512-row chunk
                    for ct in range(CT):
                        pq = PPQ.tile([128, 512], F32, tag="pq")
                        for k in range(KT):
                            nc.tensor.matmul(
                                pq[:], wqS[:, k, ct * 128:(ct + 1) * 128],
                                xTq[:, k, :], start=(k == 0), stop=(k == KT - 1))
                        nc.scalar.activation(
                            qT[:, ct, quarter * 512:(quarter + 1) * 512],
                            pq[:], AF.Identity, bias=bqS[:, ct:ct + 1], scale=1.0)
                        pk = PPQ.tile([128, 512], F32, tag="pq")
                        for k in range(KT):
                            nc.tensor.matmul(
                                pk[:], wkS[:, k, ct * 128:(ct + 1) * 128],
                                xTq[:, k, :], start=(k == 0), stop=(k == KT - 1))
                        nc.vector.tensor_copy(
                            kT[:, ct, quarter * 512:(quarter + 1) * 512], pk[:])
                    # v projection for the 4 row-tiles of this chunk
                    for r in range(4):
                        rt = quarter * 4 + r
                        pv = PPV.tile([128, C], F32, tag="pv")
                        nc.tensor.matmul(pv[:], oneRow[:], bvS[:],
                                         start=True, stop=False)
                        for k in range(KT):
                            nc.tensor.matmul(
                                pv[:], xTq[:, k, r * 128:(r + 1) * 128],
                                wvS[:, k, :], start=False, stop=(k == KT - 1))
                        # scatter heads into pair-augmented layout
                        # vA (even heads) -> block offset 0; vB (odd) -> 128
                        dstA = _ap3(vS[:, rt, 0, 0], 0,
                                    [list(vS[:].ap[0]), [192, 3], [1, 64]])
                        srcA = _ap3(pv[:, 0], 0,
                                    [list(pv[:].ap[0]), [128, 3], [1, 64]])
                        nc.vector.tensor_copy(dstA.bitcast(F32R), srcA)
                        dstB = _ap3(vS[:, rt, 0, 0], 128,
                                    [list(vS[:].ap[0]), [192, 3], [1, 64]])
                        srcB = _ap3(pv[:, 0], 64,
                                    [list(pv[:].ap[0]), [128, 3], [1, 64]])
                        nc.vector.tensor_copy(dstB.bitcast(F32R), srcB)

            # ---------------- phase 2: attention ----------------
            with tc.tile_pool(name="Epool", bufs=6) as PEP, \
                 tc.tile_pool(name="snat_sb", bufs=4) as PSN, \
                 tc.tile_pool(name="normp", bufs=2) as PNR, \
                 tc.tile_pool(name="sT_ps", bufs=2, space="PSUM") as PST, \
                 tc.tile_pool(name="ev_ps", bufs=2, space="PSUM") as PEVP, \
                 tc.tile_pool(name="snat_ps", bufs=2, space="PSUM") as PSNP:

                def emit_snat(h, qt, c):
                    cth, po = h // 2, (h % 2) * 64
                    valid = min(512, (qt + 1) * 128 - c * 512)
                    sn = PSNP.tile([128, 512], F32, tag="sn")
                    nc.tensor.matmul(
                        sn[:],
                        qT[po:po + 64, cth, qt * 128:(qt + 1) * 128],
                        kT[po:po + 64, cth, c * 512:c * 512 + 512],
                        start=True, stop=True)
                    sb = PSN.tile([128, 512], F32, tag="sb")
                    if (qt + c) % 2 == 0:
                        nc.scalar.copy(sb[:, :valid], sn[:, :valid])
                    else:
                        nc.vector.tensor_copy(sb[:, :valid], sn[:, :valid])
                    nc.sync.dma_start(
                        qk[h, qt * 128:(qt + 1) * 128, c * 512:c * 512 + valid],
                        sb[:, :valid])

                for p in range(HC // 2) if PH >= 2 and not SKIP2 else []:
                    heads = (2 * p, 2 * p + 1)
                    # queue of independent qk-output chunks for this pair
                    snat_q = [(h, qt, c) for qt in range(NT)
                              for c in range(qt // 4 + 1) for h in heads]
                    snat_i = 0

                    def drain_snat(k):
                        nonlocal snat_i
                        for _ in range(k):
                            if snat_i < len(snat_q):
                                emit_snat(*snat_q[snat_i])
                                snat_i += 1

                    for g in range(G):
                        njt = 4 * g + 4
                        evs = {}
                        for h in heads:
                            evs[h] = PEVP.tile([128, 512], F32, name=f"ev{h}", tag="ev")
                        for jp in range(njt // 2):
                            Es = {}
                            for h in heads:
                                cth, po = h // 2, (h % 2) * 64
                                st = PST.tile([128, 1024], F32, tag="st")
                                for u in range(2):
                                    jt = 2 * jp + u
                                    nc.tensor.matmul(
                                        st[:, u * 512:(u + 1) * 512],
                                        kT[po:po + 64, cth, jt * 128:(jt + 1) * 128],
                                        qT[po:po + 64, cth, g * 512:(g + 1) * 512],
                                        start=True, stop=True)
                                E = PEP.tile([128, 1024], F32R, name=f"E{h}", tag="E")
                                Es[h] = E
                                for u in range(2):
                                    jt = 2 * jp + u
                                    o = jt - 4 * g
                                    nc.scalar.activation(
                                        E[:, u * 512:(u + 1) * 512],
                                        st[:, u * 512:(u + 1) * 512], AF.Exp)
                                    if o >= 0:
                                        eng = nc.vector if MASK_DVE else nc.gpsimd
                                        eng.tensor_tensor(
                                            out=E[:, u * 512:(u + 1) * 512],
                                            in0=E[:, u * 512:(u + 1) * 512],
                                            in1=maskS[:, o, :], op=OP.mult)
                            for h in heads:
                                pair, vofs = h // 2, (0 if h % 2 == 0 else 64)
                                for u in range(2):
                                    jt = 2 * jp + u
                                    nc.tensor.matmul(
                                        evs[h][:], vS[:, jt, pair, vofs:vofs + 128],
                                        Es[h][:, u * 512:(u + 1) * 512],
                                        start=(jt == 0), stop=(jt == njt - 1))
                            drain_snat(4)
                        for h in heads:
                            cth, po = h // 2, (h % 2) * 64
                            nlo, zlo = (0, 64) if h % 2 == 0 else (64, 0)
                            rz = PNR.tile([64, 512], F32, tag="rz")
                            nc.vector.reciprocal(rz[:], evs[h][zlo:zlo + 64, :])
                            nc.vector.tensor_tensor(
                                out=attT[po:po + 64, cth, g * 512:(g + 1) * 512],
                                in0=evs[h][nlo:nlo + 64, :], in1=rz[:], op=OP.mult)
                    drain_snat(len(snat_q))

            # ---------------- phase 3: output projection ----------------
            with tc.tile_pool(name="out_sb", bufs=3) as POS, \
                 tc.tile_pool(name="wo_ps", bufs=2, space="PSUM") as PWO:
                for qt in range(NT if PH >= 3 else 0):
                    for nch, nsz in ((0, 512), (512, 256)):
                        po_ = PWO.tile([128, 512], F32, tag="po_")
                        for ctt in range(CT):
                            nc.tensor.matmul(
                                po_[:, :nsz],
                                attT[:, ctt, qt * 128:(qt + 1) * 128],
                                woS[:, ctt, nch:nch + nsz],
                                start=(ctt == 0), stop=(ctt == CT - 1))
                        ob = POS.tile([128, 512], F32, tag="ob")
                        if qt % 2 == 0:
                            nc.scalar.copy(ob[:, :nsz], po_[:, :nsz])
                        else:
                            nc.vector.tensor_copy(ob[:, :nsz], po_[:, :nsz])
                        nc.sync.dma_start(
                            out[qt * 128:(qt + 1) * 128, nch:nch + nsz],
                            ob[:, :nsz])
    nc.compile()
    return nc


def _host_prep(x, Wq, bq, Wk, Wv, bv, Wo, bo):
    """Build the 8 per-core input maps."""
    scale = float(HD) ** -0.5
    masks = np.zeros((128, 4 * 512), np.float32)
    p = np.arange(128)[:, None]
    f = np.arange(512)[None, :]
    for o in range(4):
        masks[:, o * 512:(o + 1) * 512] = (128 * o + p <= f).astype(np.float32)
    one_row = np.ones((1, 128), np.float32)

    in_maps = []
    for core in range(8):
        b, hg = core // 2, core % 2
        cols = slice(hg * C, (hg + 1) * C)
        im = {
            "xT": np.ascontiguousarray(x[b].T),
            "wq": np.ascontiguousarray(Wq[:, cols]) * np.float32(scale),
            "wk": np.ascontiguousarray(Wk[:, cols]),
            "wv": np.ascontiguousarray(Wv[:, cols]),
            "bq": (bq[cols] * np.float32(scale)).reshape(C, 1),
            "bv": bv[cols].reshape(1, C),
            "wo": np.ascontiguousarray(Wo[cols.start:cols.stop, :]),
            "masks": masks,
            "one_row": one_row,
        }
        in_maps.append(im)
    return in_maps


def kernel(x, Wq, bq, Wk, Wv, bv, Wo, bo, _profile=False):
    global _CACHED_NC
    x = np.asarray(x, np.float32)
    Wq = np.asarray(Wq, np.float32); bq = np.asarray(bq, np.float32)
    Wk = np.asarray(Wk, np.float32)
    Wv = np.asarray(Wv, np.float32); bv = np.asarray(bv, np.float32)
    Wo = np.asarray(Wo, np.float32); bo = np.asarray(bo, np.float32)

    if _CACHED_NC is None:
        _CACHED_NC = build_nc()
    nc = _CACHED_NC
    in_maps = _host_prep(x, Wq, bq, Wk, Wv, bv, Wo, bo)
    res = run_bass_kernel_spmd(nc, in_maps, core_ids=list(range(8)),
                               trace=_profile)

    output = np.empty((B, S, D), np.float32)
    qk = np.empty((B, H, S, S), np.float32)
    for core in range(8):
        b, hg = core // 2, core % 2
        r = res.results[core]
        qk[b, hg * HC:(hg + 1) * HC] = r["qk"]
        if hg == 0:
            output[b] = r["out"]
        else:
            output[b] += r["out"]
    output += bo[None, None, :]

    # causal -inf fill (constant region)
    tri = np.triu(np.ones((128, 128), dtype=bool), k=1)
    for qt in range(NT):
        r0 = qt * 128
        qk[:, :, r0:r0 + 128, r0 + 128:] = -np.inf
        blk = qk[:, :, r0:r0 + 128, r0:r0 + 128]
        blk[:, :, tri] = -np.inf
    if _profile:
        return (output, qk), res
    return output, qk


# revision 12
# speedup vs baseline: 1.1547x; 1.1547x over previous
"""Trainium2 Bass kernel for 12-head causal MHA (B=4, S=2048, D=768).

Sharding: 8 cores = 4 batches x 2 head-groups (6 heads each).
Device per core: project qT/kT (feature-major) + v (row-major, head-pair
augmented with ones cols); dual score computation:
  - natural [q, kv] pass -> qk logits output (causal-valid region only)
  - transposed [kv, q] pass -> exp -> masked -> [v|1]^T E matmul accumulating
    numerator + Z together in PSUM -> reciprocal * numerator = attn output
    directly in the transposed layout needed for the Wo projection.
All matmuls in float32r (full-rate fp32).
Host: shard prep (transpose x, slice/scale weights), sum the two Wo partial
products per batch + bo, and fill the constant causal -inf region of qk.
"""

import os
import numpy as np

import concourse.bacc as bacc
import concourse.mybir as mybir
import concourse.tile as tile
import concourse.ap as ap_mod
from concourse.bass_utils import run_bass_kernel_spmd

F32 = mybir.dt.float32
F32R = mybir.dt.float32r
AF = mybir.ActivationFunctionType
OP = mybir.AluOpType

B, D, H, HD = 4, 768, 12, 64
S = int(os.environ.get("K_S", "2048"))
HC = 6            # heads per core
C = HC * HD       # 384 cols per core
KT = D // 128     # 6 feature tiles
CT = C // 128     # 3 col tiles
NT = S // 128     # row tiles
G = S // 512      # q supertiles
NQ = S // 512     # xT quarters (512 rows each)

_CACHED_NC = None


def _ap3(base, offset, pattern):
    return ap_mod.AP(tensor=base.tensor, offset=base.offset + offset, ap=pattern)


def build_nc():
    PH = int(os.environ.get("K_PHASES", "3"))
    MASK_DVE = os.environ.get("K_MASK_DVE", "0") == "1"
    SKIP2 = os.environ.get("K_SKIP2", "0") == "1"
    nc = bacc.Bacc(trn_type="TRN2", target_bir_lowering=False, debug=False)

    xT = nc.dram_tensor("xT", [D, S], F32, kind="ExternalInput").ap()
    wq = nc.dram_tensor("wq", [D, C], F32, kind="ExternalInput").ap()
    wk = nc.dram_tensor("wk", [D, C], F32, kind="ExternalInput").ap()
    wv = nc.dram_tensor("wv", [D, C], F32, kind="ExternalInput").ap()
    bq = nc.dram_tensor("bq", [C, 1], F32, kind="ExternalInput").ap()
    bv = nc.dram_tensor("bv", [1, C], F32, kind="ExternalInput").ap()
    wo = nc.dram_tensor("wo", [C, D], F32, kind="ExternalInput").ap()
    masks = nc.dram_tensor("masks", [128, 4 * 512], F32, kind="ExternalInput").ap()
    one_row = nc.dram_tensor("one_row", [1, 128], F32, kind="ExternalInput").ap()

    qk = nc.dram_tensor("qk", [HC, S, S], F32, kind="ExternalOutput").ap()
    out = nc.dram_tensor("out", [S, D], F32, kind="ExternalOutput").ap()

    with tile.TileContext(nc) as tc:
        with tc.tile_pool(name="persist", bufs=1) as P:
            qTzE = P.tile([128, CT, S], F32R, tag="qTzE")
            qTzO = P.tile([128, CT, S], F32R, tag="qTzO")
            kT = P.tile([128, CT, S], F32R, tag="kT")
            # v per row-tile, per head-pair: [vA(64) | ones(64) | vB(64)]
            vS = P.tile([128, NT, 3, 192], F32R, tag="vS")
            attT = P.tile([128, CT, S], F32R, tag="attT")
            woS = P.tile([128, CT, D], F32R, tag="woS")
            maskS = P.tile([128, 4, 512], F32R, tag="maskS")
            bqS = P.tile([128, CT], F32, tag="bqS")
            bvS = P.tile([1, C], F32R, tag="bvS")
            oneRow = P.tile([1, 128], F32R, tag="oneRow")
            onesT = P.tile([128, 64], F32, tag="onesT")
            zeroT = P.tile([128, 64], F32, tag="zeroT")

            nc.sync.dma_start(woS[:], wo.rearrange("(ct p) d -> p ct d", p=128).bitcast(F32R))
            nc.sync.dma_start(maskS[:], masks.rearrange("p (m f) -> p m f", f=512).bitcast(F32R))
            nc.sync.dma_start(bqS[:], bq.rearrange("(ct p) one -> p (ct one)", p=128))
            nc.sync.dma_start(bvS[:], bv.bitcast(F32R))
            nc.sync.dma_start(oneRow[:], one_row.bitcast(F32R))
            nc.vector.memset(onesT[:], 1.0)
            nc.vector.memset(zeroT[:], 0.0)
            for ct in range(CT):
                zsrc = _ap3(zeroT[:, 0], 0,
                            [[list(zeroT[:].ap[0])[0], 64], [0, S // 64], [1, 64]])
                nc.vector.tensor_copy(
                    qTzE[64:128, ct, :].rearrange("p (a b) -> p a b", b=64).bitcast(F32R),
                    zsrc.bitcast(F32R))
                nc.vector.tensor_copy(
                    qTzO[0:64, ct, :].rearrange("p (a b) -> p a b", b=64).bitcast(F32R),
                    zsrc.bitcast(F32R))

            # ---------------- phase 1: projections ----------------
            with tc.tile_pool(name="xw", bufs=1) as PX, \
                 tc.tile_pool(name="wqkv", bufs=1) as PW, \
                 tc.tile_pool(name="ppq", bufs=2, space="PSUM") as PPQ, \
                 tc.tile_pool(name="ppv", bufs=2, space="PSUM") as PPV:
                wqS = PW.tile([128, KT, C], F32R, tag="wqS")
                wkS = PW.tile([128, KT, C], F32R, tag="wkS")
                wvS = PW.tile([128, KT, C], F32R, tag="wvS")
                nc.sync.dma_start(wqS[:], wq.rearrange("(k p) c -> p k c", p=128).bitcast(F32R))
                nc.sync.dma_start(wkS[:], wk.rearrange("(k p) c -> p k c", p=128).bitcast(F32R))
                nc.sync.dma_start(wvS[:], wv.rearrange("(k p) c -> p k c", p=128).bitcast(F32R))

                # ones columns of vS (positions 64..128 within each 192 block)
                for rt in range(NT):
                    dst = _ap3(vS[:, rt, 0, 0], 64,
                               [list(vS[:].ap[0]), [192, 3], [1, 64]])
                    src = _ap3(onesT[:, 0], 0,
                               [list(onesT[:].ap[0]), [0, 3], [1, 64]])
                    nc.vector.tensor_copy(dst.bitcast(F32R), src.bitcast(F32R))

                for quarter in range(NQ):
                    xTq = PX.tile([128, KT, 512], F32R, tag="xTq")
                    nc.sync.dma_start(
                        xTq[:],
                        xT[:, quarter * 512:(quarter + 1) * 512]
                        .rearrange("(k p) s -> p k s", p=128).bitcast(F32R))
                    # qT / kT projections for this 512-row chunk
                    for ct in range(CT):
                        pq = PPQ.tile([128, 512], F32, tag="pq")
                        for k in range(KT):
                            nc.tensor.matmul(
                                pq[:], wqS[:, k, ct * 128:(ct + 1) * 128],
                                xTq[:, k, :], start=(k == 0), stop=(k == KT - 1))
                        nc.scalar.activation(
                            qTzE[0:64, ct, quarter * 512:(quarter + 1) * 512],
                            pq[0:64, :], AF.Identity,
                            bias=bqS[0:64, ct:ct + 1], scale=1.0)
                        nc.scalar.activation(
                            qTzO[64:128, ct, quarter * 512:(quarter + 1) * 512],
                            pq[64:128, :], AF.Identity,
                            bias=bqS[64:128, ct:ct + 1], scale=1.0)
                        pk = PPQ.tile([128, 512], F32, tag="pq")
                        for k in range(KT):
                            nc.tensor.matmul(
                                pk[:], wkS[:, k, ct * 128:(ct + 1) * 128],
                                xTq[:, k, :], start=(k == 0), stop=(k == KT - 1))
                        nc.vector.tensor_copy(
                            kT[:, ct, quarter * 512:(quarter + 1) * 512], pk[:])
                    # v projection for the 4 row-tiles of this chunk
                    for r in range(4):
                        rt = quarter * 4 + r
                        pv = PPV.tile([128, C], F32, tag="pv")
                        nc.tensor.matmul(pv[:], oneRow[:], bvS[:],
                                         start=True, stop=False)
                        for k in range(KT):
                            nc.tensor.matmul(
                                pv[:], xTq[:, k, r * 128:(r + 1) * 128],
                                wvS[:, k, :], start=False, stop=(k == KT - 1))
                        # scatter heads into pair-augmented layout
                        # vA (even heads) -> block offset 0; vB (odd) -> 128
                        dstA = _ap3(vS[:, rt, 0, 0], 0,
                                    [list(vS[:].ap[0]), [192, 3], [1, 64]])
                        srcA = _ap3(pv[:, 0], 0,
                                    [list(pv[:].ap[0]), [128, 3], [1, 64]])
                        nc.vector.tensor_copy(dstA.bitcast(F32R), srcA)
                        dstB = _ap3(vS[:, rt, 0, 0], 128,
                                    [list(vS[:].ap[0]), [192, 3], [1, 64]])
                        srcB = _ap3(pv[:, 0], 64,
                                    [list(pv[:].ap[0]), [128, 3], [1, 64]])
                        nc.vector.tensor_copy(dstB.bitcast(F32R), srcB)

            # ---------------- phase 2: attention ----------------
            with tc.tile_pool(name="Epool", bufs=6) as PEP, \
                 tc.tile_pool(name="snat_sb", bufs=4) as PSN, \
                 tc.tile_pool(name="normp", bufs=2) as PNR, \
                 tc.tile_pool(name="sT_ps", bufs=2, space="PSUM") as PST, \
                 tc.tile_pool(name="ev_ps", bufs=2, space="PSUM") as PEVP, \
                 tc.tile_pool(name="snat_ps", bufs=2, space="PSUM") as PSNP:

                def emit_snat(h, qt, c):
                    cth = h // 2
                    qTz = qTzE if h % 2 == 0 else qTzO
                    valid = min(512, (qt + 1) * 128 - c * 512)
                    sn = PSNP.tile([128, 512], F32, tag="sn")
                    nc.tensor.matmul(
                        sn[:],
                        qTz[:, cth, qt * 128:(qt + 1) * 128],
                        kT[:, cth, c * 512:c * 512 + 512],
                        start=True, stop=True)
                    sb = PSN.tile([128, 512], F32, tag="sb")
                    if (qt + c) % 2 == 0:
                        nc.scalar.copy(sb[:, :valid], sn[:, :valid])
                    else:
                        nc.vector.tensor_copy(sb[:, :valid], sn[:, :valid])
                    nc.sync.dma_start(
                        qk[h, qt * 128:(qt + 1) * 128, c * 512:c * 512 + valid],
                        sb[:, :valid])

                for p in range(HC // 2) if PH >= 2 and not SKIP2 else []:
                    heads = (2 * p, 2 * p + 1)
                    # queue of independent qk-output chunks for this pair
                    snat_q = [(h, qt, c) for qt in range(NT)
                              for c in range(qt // 4 + 1) for h in heads]
                    snat_i = 0

                    def drain_snat(k):
                        nonlocal snat_i
                        for _ in range(k):
                            if snat_i < len(snat_q):
                                emit_snat(*snat_q[snat_i])
                                snat_i += 1

                    for g in range(G):
                        njt = 4 * g + 4
                        evs = {}
                        for h in heads:
                            evs[h] = PEVP.tile([128, 512], F32, name=f"ev{h}", tag="ev")
                        for jp in range(njt // 2):
                            Es = {}
                            for h in heads:
                                cth = h // 2
                                qTz = qTzE if h % 2 == 0 else qTzO
                                st = PST.tile([128, 1024], F32, tag="st")
                                for u in range(2):
                                    jt = 2 * jp + u
                                    nc.tensor.matmul(
                                        st[:, u * 512:(u + 1) * 512],
                                        kT[:, cth, jt * 128:(jt + 1) * 128],
                                        qTz[:, cth, g * 512:(g + 1) * 512],
                                        start=True, stop=True)
                                E = PEP.tile([128, 1024], F32R, name=f"E{h}", tag="E")
                                Es[h] = E
                                for u in range(2):
                                    jt = 2 * jp + u
                                    o = jt - 4 * g
                                    nc.scalar.activation(
                                        E[:, u * 512:(u + 1) * 512],
                                        st[:, u * 512:(u + 1) * 512], AF.Exp)
                                    if o >= 0:
                                        eng = nc.vector if MASK_DVE else nc.gpsimd
                                        eng.tensor_tensor(
                                            out=E[:, u * 512:(u + 1) * 512],
                                            in0=E[:, u * 512:(u + 1) * 512],
                                            in1=maskS[:, o, :], op=OP.mult)
                            for h in heads:
                                pair, vofs = h // 2, (0 if h % 2 == 0 else 64)
                                for u in range(2):
                                    jt = 2 * jp + u
                                    nc.tensor.matmul(
                                        evs[h][:], vS[:, jt, pair, vofs:vofs + 128],
                                        Es[h][:, u * 512:(u + 1) * 512],
                                        start=(jt == 0), stop=(jt == njt - 1))
                            drain_snat(4)
                        for h in heads:
                            cth, po = h // 2, (h % 2) * 64
                            nlo, zlo = (0, 64) if h % 2 == 0 else (64, 0)
                            rz = PNR.tile([64, 512], F32, tag="rz")
                            nc.vector.reciprocal(rz[:], evs[h][zlo:zlo + 64, :])
                            nc.vector.tensor_tensor(
                                out=attT[po:po + 64, cth, g * 512:(g + 1) * 512],
                                in0=evs[h][nlo:nlo + 64, :], in1=rz[:], op=OP.mult)
                    drain_snat(len(snat_q))

            # ---------------- phase 3: output projection ----------------
            with tc.tile_pool(name="out_sb", bufs=3) as POS, \
                 tc.tile_pool(name="wo_ps", bufs=2, space="PSUM") as PWO:
                for qt in range(NT if PH >= 3 else 0):
                    for nch, nsz in ((0, 512), (512, 256)):
                        po_ = PWO.tile([128, 512], F32, tag="po_")
                        for ctt in range(CT):
                            nc.tensor.matmul(
                                po_[:, :nsz],
                                attT[:, ctt, qt * 128:(qt + 1) * 128],
                                woS[:, ctt, nch:nch + nsz],
                                start=(ctt == 0), stop=(ctt == CT - 1))
                        ob = POS.tile([128, 512], F32, tag="ob")
                        if qt % 2 == 0:
                            nc.scalar.copy(ob[:, :nsz], po_[:, :nsz])
                        else:
                            nc.vector.tensor_copy(ob[:, :nsz], po_[:, :nsz])
                        nc.sync.dma_start(
                            out[qt * 128:(qt + 1) * 128, nch:nch + nsz],
                            ob[:, :nsz])
    nc.compile()
    return nc


def _host_prep(x, Wq, bq, Wk, Wv, bv, Wo, bo):
    """Build the 8 per-core input maps."""
    scale = float(HD) ** -0.5
    masks = np.zeros((128, 4 * 512), np.float32)
    p = np.arange(128)[:, None]
    f = np.arange(512)[None, :]
    for o in range(4):
        masks[:, o * 512:(o + 1) * 512] = (128 * o + p <= f).astype(np.float32)
    one_row = np.ones((1, 128), np.float32)

    in_maps = []
    for core in range(8):
        b, hg = core // 2, core % 2
        cols = slice(hg * C, (hg + 1) * C)
        im = {
            "xT": np.ascontiguousarray(x[b].T),
            "wq": np.ascontiguousarray(Wq[:, cols]) * np.float32(scale),
            "wk": np.ascontiguousarray(Wk[:, cols]),
            "wv": np.ascontiguousarray(Wv[:, cols]),
            "bq": (bq[cols] * np.float32(scale)).reshape(C, 1),
            "bv": bv[cols].reshape(1, C),
            "wo": np.ascontiguousarray(Wo[cols.start:cols.stop, :]),
            "masks": masks,
            "one_row": one_row,
        }
        in_maps.append(im)
    return in_maps


def kernel(x, Wq, bq, Wk, Wv, bv, Wo, bo, _profile=False):
    global _CACHED_NC
    x = np.asarray(x, np.float32)
    Wq = np.asarray(Wq, np.float32); bq = np.asarray(bq, np.float32)
    Wk = np.asarray(Wk, np.float32)
    Wv = np.asarray(Wv, np.float32); bv = np.asarray(bv, np.float32)
    Wo = np.asarray(Wo, np.float32); bo = np.asarray(bo, np.float32)

    if _CACHED_NC is None:
        _CACHED_NC = build_nc()
    nc = _CACHED_NC
    in_maps = _host_prep(x, Wq, bq, Wk, Wv, bv, Wo, bo)
    res = run_bass_kernel_spmd(nc, in_maps, core_ids=list(range(8)),
                               trace=_profile)

    output = np.empty((B, S, D), np.float32)
    qk = np.empty((B, H, S, S), np.float32)
    for core in range(8):
        b, hg = core // 2, core % 2
        r = res.results[core]
        qk[b, hg * HC:(hg + 1) * HC] = r["qk"]
        if hg == 0:
            output[b] = r["out"]
        else:
            output[b] += r["out"]
    output += bo[None, None, :]

    # causal -inf fill (constant region)
    tri = np.triu(np.ones((128, 128), dtype=bool), k=1)
    for qt in range(NT):
        r0 = qt * 128
        qk[:, :, r0:r0 + 128, r0 + 128:] = -np.inf
        blk = qk[:, :, r0:r0 + 128, r0:r0 + 128]
        blk[:, :, tri] = -np.inf
    if _profile:
        return (output, qk), res
    return output, qk
